# revision 1
# baseline (speedup 1.0000x reference)
"""Trainium2 Bass kernel for nn_CustomLoss_82257213653439.

Computes: mean_i( -w_i * log(outputs[i, targets[i]]) ) with
w_i = 0.7 if targets[i] != 0 else 0.3, over outputs [1048576, 128] f32.

Data-parallel over 8 cores (N-axis sharding), R = N/8 = 131072 rows/core.

Per-core algorithm (matmul-trace):
  L = Ln(X)                   ACT, bf16 out, streamed in 16 chunks
  M_r[p,c] = (t[row] == c)    one-hot per 128-row tile, DVE tensor_scalar
  G += M_r^T @ L_r            PE, accumulated in one PSUM tile [128,128]
Then G[c,c'] = sum_{rows: t=c} log(x[row,c']), so
  S_all = trace(G) = sum_rows log(picked)
  S_0   = G[0,0]   = sum_{rows: t=0} log(picked)
  loss  = -(0.7*S_all - 0.4*S_0) / N     (host combines the 8 G matrices)

Layout: partition p owns rows [p*K, (p+1)*K), K = R/128; each chunk DMA
reads one contiguous 32 KB block per partition. Ln and the matmul group
are split 4x per chunk to shorten the pipeline tail. Standard instructions
only (this walrus build rejects the custom Ant ISA ops).

Measured on TRN2 HW: 194.8-199.2 us steady-state per pass (k-loop delta,
8 cores in parallel); TimelineSim single-shot estimate 203.5 us. The cost
model was anchored against HW twice (219.0 us vs 218.6 us predicted on the
previous revision; 199.2 us vs 203.5 us here). Pure-stream DMA floor for
the 64 MB/core read at 358 GB/s is ~179 us, so this sits at ~1.1x roofline.
"""

import ml_dtypes
import numpy as np

import concourse.bass as bass
from concourse import mybir
from concourse.bass_utils import run_bass_kernel_spmd

N, C = 1048576, 128
NCORES = 8
P = 128
SWING = 0.7

F32 = mybir.dt.float32
BF16 = mybir.dt.bfloat16
BF = ml_dtypes.bfloat16


def _build_nc(R=N // NCORES, nchunk=16, asp=4, psub=4):
    NT = R // P          # 128-row tiles per core (= K, rows per partition)
    K = R // P
    TC = NT // nchunk    # tiles per chunk

    nc = bass.Bass(target_bir_lowering=False)
    x = nc.dram_tensor("x", [R * C], F32, kind="ExternalInput")
    tgt = nc.dram_tensor("tgt", [P, NT], F32, kind="ExternalInput")
    iot = nc.dram_tensor("iot", [P, C], BF16, kind="ExternalInput")
    gout = nc.dram_tensor("g", [P, C], F32, kind="ExternalOutput")

    with (
        nc.sbuf_tensor("t_sb", [P, NT], F32) as t_sb,
        nc.sbuf_tensor("iota_sb", [P, C], BF16) as iota_sb,
        nc.sbuf_tensor("x_sb", [P, 2, TC, C], F32) as x_sb,
        nc.sbuf_tensor("l_sb", [P, 2, TC, C], BF16) as l_sb,
        nc.sbuf_tensor("m_sb", [P, 2, TC, C], BF16) as m_sb,
        nc.sbuf_tensor("g_sb", [P, C], F32) as g_sb,
        nc.psum_tensor("g_ps", [P, C], F32) as g_ps,
        nc.semaphore("cin") as cin,        # t/iota input DMAs
        nc.semaphore("xin0") as xin0,      # x chunk DMAs (even chunks)
        nc.semaphore("xin1") as xin1,      # x chunk DMAs (odd chunks)
        nc.semaphore("act_done") as act_done,
        nc.semaphore("dve_done") as dve_done,
        nc.semaphore("pe_done") as pe_done,
        nc.semaphore("g_done") as g_done,
        nc.semaphore("outsem") as outsem,
        nc.Block() as block,
    ):

        @block.sync
        def _(sync):
            sync.dma_start(out=t_sb[:], in_=tgt[:]).then_inc(cin, 16)
            sync.dma_start(out=iota_sb[:], in_=iot[:]).then_inc(cin, 16)
            sync.wait_ge(g_done, 1)
            sync.dma_start(out=gout[:], in_=g_sb[:]).then_inc(outsem, 16)
            sync.wait_ge(outsem, 16)

        @block.gpsimd
        def _(gpsimd):
            for i in range(nchunk):
                b = i % 2
                if i >= 2:
                    # WAR: ACT must have fully read x_sb[b] (chunk i-2)
                    gpsimd.wait_ge(act_done, (i - 1) * asp)
                # chunk i: per partition one contiguous TC*C block
                src = bass.AP(x, i * TC * C, [[K * C, P], [1, TC * C]])
                gpsimd.dma_start(out=x_sb[:, b, :, :], in_=src).then_inc(
                    xin0 if b == 0 else xin1, 16
                )

        @block.scalar
        def _(scalar):
            sub = TC // asp
            for i in range(nchunk):
                b = i % 2
                scalar.wait_ge(xin0 if b == 0 else xin1, (i // 2 + 1) * 16)
                if i >= 2:
                    # WAR: PE must have consumed l_sb[b] (chunk i-2)
                    scalar.wait_ge(pe_done, (i - 1) * psub)
                for s in range(asp):
                    scalar.activation(
                        out=l_sb[:, b, s * sub : (s + 1) * sub, :].rearrange(
                            "p t c -> p (t c)"
                        ),
                        in_=x_sb[:, b, s * sub : (s + 1) * sub, :].rearrange(
                            "p t c -> p (t c)"
                        ),
                        func=mybir.ActivationFunctionType.Ln,
                    ).then_inc(act_done, 1)

        @block.vector
        def _(vector):
            vector.wait_ge(cin, 32)
            for i in range(nchunk):
                b = i % 2
                if i >= 2:
                    vector.wait_ge(pe_done, (i - 1) * psub)
                last = None
                for r in range(TC):
                    last = vector.tensor_scalar(
                        out=m_sb[:, b, r, :],
                        in0=iota_sb[:],
                        scalar1=t_sb[:, i * TC + r : i * TC + r + 1],
                        scalar2=None,
                        op0=mybir.AluOpType.is_equal,
                    )
                last.then_inc(dve_done, 1)
            vector.wait_ge(pe_done, nchunk * psub)
            vector.tensor_copy(out=g_sb[:], in_=g_ps[:]).then_inc(g_done, 1)

        @block.tensor
        def _(tensor):
            sub = TC // psub
            for i in range(nchunk):
                b = i % 2
                tensor.wait_ge(dve_done, i + 1)
                for s in range(psub):
                    # asp == psub: Ln sub-op s covers exactly these tiles
                    tensor.wait_ge(act_done, i * asp + s + 1)
                    last = None
                    for rr in range(sub):
                        r = s * sub + rr
                        g = i * TC + r
                        last = nc.tensor.matmul(
                            out=g_ps[:],
                            lhsT=m_sb[:, b, r, :],
                            rhs=l_sb[:, b, r, :],
                            start=(g == 0),
                            stop=(g == NT - 1),
                        )
                    last.then_inc(pe_done, 1)

    return nc


_NC_CACHE = None


def _get_nc():
    global _NC_CACHE
    if _NC_CACHE is None:
        _NC_CACHE = _build_nc()
    return _NC_CACHE


def _make_in_maps(outputs, targets, R=N // NCORES):
    K = R // P
    iota = np.broadcast_to(np.arange(C, dtype=np.float32), (P, C)).astype(BF)
    in_maps = []
    for i in range(NCORES):
        sl = slice(i * R, (i + 1) * R)
        xs = np.ascontiguousarray(outputs[sl]).reshape(R * C)
        # partition p owns rows [p*K, (p+1)*K): t_sb[p, j] = t[p*K + j]
        ts = np.ascontiguousarray(targets[sl]).astype(np.float32).reshape(P, K)
        in_maps.append({"x": xs, "tgt": ts, "iot": iota})
    return in_maps


def _combine(results):
    s_all = 0.0
    s0 = 0.0
    for r in results:
        g = r["g"].astype(np.float64)
        s_all += np.trace(g)
        s0 += g[0, 0]
    wsum = SWING * s_all - (2 * SWING - 1.0) * s0
    return np.float32(-wsum / N)


def kernel(outputs, targets, _trace=False, **_kw):
    nc = _get_nc()
    in_maps = _make_in_maps(np.asarray(outputs), np.asarray(targets))
    res = run_bass_kernel_spmd(
        nc, in_maps, core_ids=list(range(NCORES)), trace=_trace
    )
    out = _combine(res.results)
    if _trace:
        return out, res
    return out



# revision 3
# speedup vs baseline: 1.1416x; 1.1416x over previous
"""Trainium2 Bass kernel for nn_CustomLoss_82257213653439.

Computes: mean_i( -w_i * log(outputs[i, targets[i]]) ) with
w_i = 0.7 if targets[i] != 0 else 0.3, over outputs [1048576, 128] f32.

Data-parallel over 8 cores (N-axis sharding), R = N/8 = 131072 rows/core.

Per-core algorithm (matmul-trace):
  L = Ln(X)                   ACT, bf16 out, streamed in 16 chunks
  M_r[p,c] = (t[row] == c)    one-hot per 128-row tile, DVE tensor_scalar
  G += M_r^T @ L_r            PE, accumulated in one PSUM tile [128,128]
Then G[c,c'] = sum_{rows: t=c} log(x[row,c']), so
  S_all = trace(G) = sum_rows log(picked)
  S_0   = G[0,0]   = sum_{rows: t=0} log(picked)
  loss  = -(0.7*S_all - 0.4*S_0) / N     (host combines the 8 G matrices)

Layout: partition p owns rows [p*K, (p+1)*K), K = R/128; each chunk DMA
reads one contiguous 32 KB block per partition. Ln and the matmul group
are split 4x per chunk to shorten the pipeline tail. Standard instructions
only (this walrus build rejects the custom Ant ISA ops).

Measured on TRN2 HW: 194.8-199.2 us steady-state per pass (k-loop delta,
8 cores in parallel); TimelineSim single-shot estimate 203.5 us. The cost
model was anchored against HW twice (219.0 us vs 218.6 us predicted on the
previous revision; 199.2 us vs 203.5 us here). Pure-stream DMA floor for
the 64 MB/core read at 358 GB/s is ~179 us, so this sits at ~1.1x roofline.
"""

import ml_dtypes
import numpy as np

import concourse.bass as bass
from concourse import mybir
from concourse.bass_utils import run_bass_kernel_spmd

N, C = 1048576, 128
NCORES = 8
P = 128
SWING = 0.7

F32 = mybir.dt.float32
BF16 = mybir.dt.bfloat16
BF = ml_dtypes.bfloat16


def _build_nc(R=N // NCORES, nchunk=16, asp=4, psub=4, bufs=3):
    NT = R // P          # 128-row tiles per core (= K, rows per partition)
    K = R // P
    TC = NT // nchunk    # tiles per chunk

    nc = bass.Bass(target_bir_lowering=False)
    x = nc.dram_tensor("x", [R * C], F32, kind="ExternalInput")
    tgt = nc.dram_tensor("tgt", [P, NT], F32, kind="ExternalInput")
    iot = nc.dram_tensor("iot", [P, C], BF16, kind="ExternalInput")
    gout = nc.dram_tensor("g", [P, C], F32, kind="ExternalOutput")

    with (
        nc.sbuf_tensor("t_sb", [P, NT], F32) as t_sb,
        nc.sbuf_tensor("iota_sb", [P, C], BF16) as iota_sb,
        nc.sbuf_tensor("x_sb", [P, bufs, TC, C], F32) as x_sb,
        nc.sbuf_tensor("l_sb", [P, bufs, TC, C], BF16) as l_sb,
        nc.sbuf_tensor("m_sb", [P, bufs, TC, C], BF16) as m_sb,
        nc.sbuf_tensor("g_sb", [P, C], F32) as g_sb,
        nc.psum_tensor("g_ps", [P, C], F32) as g_ps,
        nc.semaphore("cin") as cin,        # t/iota input DMAs
        nc.semaphore("xin0") as xin0,      # x chunk DMAs (even chunks)
        nc.semaphore("xin1") as xin1,      # x chunk DMAs (odd chunks)
        nc.semaphore("act_done") as act_done,
        nc.semaphore("dve_done") as dve_done,
        nc.semaphore("pe_done") as pe_done,
        nc.semaphore("g_done") as g_done,
        nc.semaphore("outsem") as outsem,
        nc.Block() as block,
    ):

        @block.sync
        def _(sync):
            sync.dma_start(out=t_sb[:], in_=tgt[:]).then_inc(cin, 16)
            sync.dma_start(out=iota_sb[:], in_=iot[:]).then_inc(cin, 16)
            sync.wait_ge(g_done, 1)
            sync.dma_start(out=gout[:], in_=g_sb[:]).then_inc(outsem, 16)
            sync.wait_ge(outsem, 16)

        @block.gpsimd
        def _(gpsimd):
            for i in range(nchunk):
                b = i % bufs
                if i >= bufs:
                    # WAR: ACT must have fully read x_sb[b] (chunk i-bufs)
                    gpsimd.wait_ge(act_done, (i - bufs + 1) * asp)
                # chunk i: per partition one contiguous TC*C block
                src = bass.AP(x, i * TC * C, [[K * C, P], [1, TC * C]])
                gpsimd.dma_start(out=x_sb[:, b, :, :], in_=src).then_inc(
                    xin0 if i % 2 == 0 else xin1, 16
                )

        @block.scalar
        def _(scalar):
            sub = TC // asp
            for i in range(nchunk):
                b = i % bufs
                scalar.wait_ge(xin0 if i % 2 == 0 else xin1, (i // 2 + 1) * 16)
                if i >= bufs:
                    # WAR: PE must have consumed l_sb[b] (chunk i-bufs)
                    scalar.wait_ge(pe_done, (i - bufs + 1) * psub)
                for s in range(asp):
                    scalar.activation(
                        out=l_sb[:, b, s * sub : (s + 1) * sub, :].rearrange(
                            "p t c -> p (t c)"
                        ),
                        in_=x_sb[:, b, s * sub : (s + 1) * sub, :].rearrange(
                            "p t c -> p (t c)"
                        ),
                        func=mybir.ActivationFunctionType.Ln,
                    ).then_inc(act_done, 1)

        @block.vector
        def _(vector):
            vector.wait_ge(cin, 32)
            for i in range(nchunk):
                b = i % bufs
                if i >= bufs:
                    vector.wait_ge(pe_done, (i - bufs + 1) * psub)
                last = None
                for r in range(TC):
                    last = vector.tensor_scalar(
                        out=m_sb[:, b, r, :],
                        in0=iota_sb[:],
                        scalar1=t_sb[:, i * TC + r : i * TC + r + 1],
                        scalar2=None,
                        op0=mybir.AluOpType.is_equal,
                    )
                last.then_inc(dve_done, 1)
            vector.wait_ge(pe_done, nchunk * psub)
            vector.tensor_copy(out=g_sb[:], in_=g_ps[:]).then_inc(g_done, 1)

        @block.tensor
        def _(tensor):
            sub = TC // psub
            for i in range(nchunk):
                b = i % bufs
                tensor.wait_ge(dve_done, i + 1)
                for s in range(psub):
                    # asp == psub: Ln sub-op s covers exactly these tiles
                    tensor.wait_ge(act_done, i * asp + s + 1)
                    last = None
                    for rr in range(sub):
                        r = s * sub + rr
                        g = i * TC + r
                        last = nc.tensor.matmul(
                            out=g_ps[:],
                            lhsT=m_sb[:, b, r, :],
                            rhs=l_sb[:, b, r, :],
                            start=(g == 0),
                            stop=(g == NT - 1),
                        )
                    last.then_inc(pe_done, 1)

    return nc


_NC_CACHE = None


def _get_nc():
    global _NC_CACHE
    if _NC_CACHE is None:
        _NC_CACHE = _build_nc()
    return _NC_CACHE


def _make_in_maps(outputs, targets, R=N // NCORES):
    K = R // P
    iota = np.broadcast_to(np.arange(C, dtype=np.float32), (P, C)).astype(BF)
    in_maps = []
    for i in range(NCORES):
        sl = slice(i * R, (i + 1) * R)
        xs = np.ascontiguousarray(outputs[sl]).reshape(R * C)
        # partition p owns rows [p*K, (p+1)*K): t_sb[p, j] = t[p*K + j]
        ts = np.ascontiguousarray(targets[sl]).astype(np.float32).reshape(P, K)
        in_maps.append({"x": xs, "tgt": ts, "iot": iota})
    return in_maps


def _combine(results):
    s_all = 0.0
    s0 = 0.0
    for r in results:
        g = r["g"].astype(np.float64)
        s_all += np.trace(g)
        s0 += g[0, 0]
    wsum = SWING * s_all - (2 * SWING - 1.0) * s0
    return np.float32(-wsum / N)


def kernel(outputs, targets, _trace=False, **_kw):
    nc = _get_nc()
    in_maps = _make_in_maps(np.asarray(outputs), np.asarray(targets))
    res = run_bass_kernel_spmd(
        nc, in_maps, core_ids=list(range(NCORES)), trace=_trace
    )
    out = _combine(res.results)
    if _trace:
        return out, res
    return out

